# revision 1
# baseline (speedup 1.0000x reference)
"""Trainium2 Bass kernel for nn_CoefficientDecoder.

reference computation (all f32):
    h = relu(x @ W1.T + b1)         x:[B,256] -> h:[B,64]
    h = h @ Wd3.T + bd3             [B,64]
    h = h @ Wd2.T + bd2             [B,64]
    h = h @ Wd1.T + bd1             [B,64]
    z = h @ W2.T + b2               [B,512]
    out = z @ bases                 bases:[512,4096] -> out:[B,4096]

Strategy: pure data-parallel over the batch dim across 8 NeuronCores
(B=8192 -> 1024 rows/core); weights + bases replicated per core.

Per-core kernel works in "transposed activation" space: the host passes
xT = x_shard.T so every matmul has the contraction dim on partitions with
weights stationary and the batch streaming as the moving operand.  All
matmuls use the fp32r (replicated-fp32) PE mode: 1 cycle/row for moving
dims >= 256 (4x faster than plain fp32) at ~1.5e-4 element precision;
the big GEMM can optionally run in fp16.  Walrus requires fp32r matmul
operands to be produced by a rounding op, so DMA-loaded fp32 tiles pass
through a DVE copy into fp32r tiles, and ACT writes h/zT directly as
fp32r.

All small constants (weights + biases) are packed host-side into one
[128, 840] tensor -> a single DMA instead of 10 (each DMA pays ~1.3 us
issue latency on the queue, which showed up as a 14 us PE startup stall).

    MLP:  hT = W1 @ xT (K=256, 2 acc steps) -> relu+bias on ACT
          dec3/dec2/dec1: 64x64 matmuls, bias via ACT Identity
          zT[512,1024] = W2 @ hT, bias fused into the PSUM->SBUF copy
    GEMM: out[mm*128:+128, s*512:+512] = sum_c zT[c].T @ bases[c, s-chunk]
          s-outer loop order so each seq-block only needs its own bases
          tile (bases DMAs stream in behind the compute); 64 output
          tiles/core, 4 matmuls each, DVE/ACT copy to SBUF, stores
          alternate between the SP and ACT HWDGE queues.

`repeat` wraps the whole body in a hardware For_i loop — used only for
timing (amortizes the ~100 ms axon dispatch overhead).
"""

import numpy as np

import concourse.bass as bass
import concourse.tile as tile
from concourse import bacc, mybir
from concourse.bass import ts
from concourse.bass_utils import run_bass_kernel_spmd

N_CORES = 8
B, IN_F, HID, NB, SEQ = 8192, 256, 64, 512, 4096
B_LOC = B // N_CORES            # 1024 batch rows per core

F32 = mybir.dt.float32
F32R = mybir.dt.float32r
F16 = mybir.dt.float16

# packed-constant column layout (fp32 columns in the [128, NCONST] tensor).
# The three dec layers are linear, so they are folded host-side into W2:
#   W2eff = W2@Wd1@Wd2@Wd3,  b2eff = b2 + (bd3@Wd2.T@Wd1.T + bd2@Wd1.T + bd1)@W2.T
C_W1K0, C_W1K1 = 0, 64
C_W2 = 128            # [64, 512] on partitions 0..63
C_B2P = 640           # [128, 4]
C_B1 = 644
NCONST = 645
NWROUND = 640         # leading region that gets rounded to fp32r

# dtype knobs: ("f32r"|"f16") for the big GEMM operands, ("f32"|"f16") output
GEMM_MODE = "f32r"
OUT_MODE = "f32"

_CACHE = {}


def _build(gemm_mode: str, out_mode: str, repeat: int = 1):
    gemm_dt = F32R if gemm_mode == "f32r" else F16    # on-chip GEMM operand dtype
    bases_dram_dt = F32 if gemm_mode == "f32r" else F16
    out_dt = F32 if out_mode == "f32" else F16

    nc = bacc.Bacc(
        "TRN2",
        target_bir_lowering=False,
        debug=False,
        enable_asserts=False,
        num_devices=N_CORES,
    )

    xT_d = nc.declare_dram_parameter("xT", [IN_F, B_LOC], F32, isOutput=False)
    consts_d = nc.declare_dram_parameter("consts", [128, NCONST], F32, isOutput=False)
    bases_d = nc.declare_dram_parameter("bases", [NB, SEQ], bases_dram_dt, isOutput=False)
    out_d = nc.declare_dram_parameter("out", [B_LOC, SEQ], out_dt, isOutput=True)

    KC = IN_F // 128        # 2 k-chunks for layer 1
    ZC = NB // 128          # 4 z-feature chunks
    NJ = B_LOC // 512       # 2 batch chunks for the MLP moving dim
    MM = B_LOC // 128       # 8 batch sub-chunks for the final GEMM
    SC = SEQ // 512         # 8 seq chunks

    relu = mybir.ActivationFunctionType.Relu
    ident = mybir.ActivationFunctionType.Identity
    copyf = mybir.ActivationFunctionType.Copy

    with tile.TileContext(nc) as tc:
        with (
            tc.tile_pool(name="const", bufs=1) as constp,
            tc.tile_pool(name="stage", bufs=2) as stagep,
            tc.tile_pool(name="bases", bufs=1) as basesp,
            tc.tile_pool(name="xz", bufs=1) as xzp,
            tc.tile_pool(name="act", bufs=3) as actp,
            tc.tile_pool(name="outsb", bufs=6) as outsbp,
            tc.tile_pool(name="mlp_ps", bufs=2, space="PSUM") as mlpp,
            tc.tile_pool(name="out_ps", bufs=6, space="PSUM") as outpp,
        ):
            def body():
                # ---- startup order tuned for the first MLP matmul: it needs
                # x half k=0 plus the packed consts; x half k=1 can trail ----
                xT_pkn = xT_d.rearrange("(k p) n -> p k n", p=128)
                xf = stagep.tile([128, KC, B_LOC], F32, tag="xstage")
                xT_sb = xzp.tile([128, KC, B_LOC], F32R, tag="xT")
                craw = constp.tile([128, NCONST], F32, tag="craw")
                crnd = constp.tile([128, NWROUND], F32R, tag="crnd")

                nc.scalar.dma_start(xf[:, 0, :], xT_pkn[:, 0, :])
                nc.vector.tensor_copy(xT_sb[:, 0, :], xf[:, 0, :])
                nc.scalar.dma_start(craw[:], consts_d[:])
                nc.vector.tensor_copy(crnd[:], craw[:, :NWROUND])
                nc.scalar.dma_start(xf[:, 1, :], xT_pkn[:, 1, :])
                nc.vector.tensor_copy(xT_sb[:, 1, :], xf[:, 1, :])

                w1t = (crnd[:, C_W1K0 : C_W1K0 + 64], crnd[:, C_W1K1 : C_W1K1 + 64])
                w2t = crnd[:HID, C_W2 : C_W2 + NB]
                b2p = craw[:, C_B2P : C_B2P + ZC]
                b1 = craw[:HID, C_B1 : C_B1 + 1]

                # ---- bases load on the SP queue: one tile per seq chunk so
                # each final-GEMM s-block only waits for its own chunk ----
                bases_pcn = bases_d.rearrange("(c p) n -> p c n", p=128)
                bases_sb = []
                for s in range(SC):
                    t = basesp.tile([128, ZC, 512], gemm_dt, tag=f"bases{s}")
                    if gemm_mode == "f32r":
                        f = stagep.tile([128, ZC, 512], F32, tag="bstage")
                        nc.sync.dma_start(f[:], bases_pcn[:, :, ts(s, 512)])
                        nc.vector.tensor_copy(t[:], f[:])
                    else:
                        nc.sync.dma_start(t[:], bases_pcn[:, :, ts(s, 512)])
                    bases_sb.append(t)

                # ---- MLP: produce zT [feature-part, ZC, batch] ----
                zT_sb = xzp.tile([128, ZC, B_LOC], gemm_dt, tag="zT")
                for j in range(NJ):
                    hp = mlpp.tile([HID, 512], F32, tag="mlp")
                    for k in range(KC):
                        nc.tensor.matmul(
                            hp[:],
                            w1t[k],
                            xT_sb[:, k, ts(j, 512)],
                            start=(k == 0),
                            stop=(k == KC - 1),
                        )
                    h = actp.tile([HID, 512], F32R, tag="h")
                    nc.scalar.activation(h[:], hp[:], relu, bias=b1)

                    for c in range(ZC):
                        zp = mlpp.tile([128, 512], F32, tag="mlp")
                        nc.tensor.matmul(
                            zp[:], w2t[:, ts(c, 128)], h[:], start=True, stop=True
                        )
                        nc.scalar.activation(
                            zT_sb[:, c, ts(j, 512)], zp[:], ident,
                            bias=b2p[:, c : c + 1],
                        )

                # ---- final GEMM: out = z @ bases (s outer: each block only
                # needs bases chunk s, which streams in behind compute) ----
                for s in range(SC):
                    for mm_i in range(MM):
                        op = outpp.tile([128, 512], F32, tag="op")
                        for c in range(ZC):
                            nc.tensor.matmul(
                                op[:],
                                zT_sb[:, c, ts(mm_i, 128)],
                                bases_sb[s][:, c, :],
                                start=(c == 0),
                                stop=(c == ZC - 1),
                            )
                        ob = outsbp.tile([128, 512], out_dt, tag="ob")
                        if (mm_i + s) % 2 == 0:
                            nc.vector.tensor_copy(ob[:], op[:])
                        else:
                            nc.scalar.activation(ob[:], op[:], copyf)
                        dma_eng = nc.sync if (mm_i % 2 == 0) else nc.scalar
                        dma_eng.dma_start(out_d[ts(mm_i, 128), ts(s, 512)], ob[:])

            if repeat == 1:
                body()
            else:
                with tc.For_i(0, repeat, 1):
                    body()

    nc.compile()
    return nc


def _get_nc(gemm_mode: str, out_mode: str, repeat: int = 1):
    key = (gemm_mode, out_mode, repeat)
    if key not in _CACHE:
        _CACHE[key] = _build(gemm_mode, out_mode, repeat)
    return _CACHE[key]


def _pack_consts(W1, b1, Wd1, bd1, Wd2, bd2, Wd3, bd3, W2, b2):
    W2eff = W2 @ Wd1 @ Wd2 @ Wd3                      # [512, 64]
    b2eff = b2 + (bd3 @ Wd2.T @ Wd1.T + bd2 @ Wd1.T + bd1) @ W2.T
    c = np.zeros((128, NCONST), np.float32)
    W1T = W1.T  # [256, 64]
    c[:, C_W1K0 : C_W1K0 + 64] = W1T[:128]
    c[:, C_W1K1 : C_W1K1 + 64] = W1T[128:]
    c[:HID, C_W2 : C_W2 + NB] = W2eff.T
    c[:, C_B2P : C_B2P + NB // 128] = b2eff.reshape(NB // 128, 128).T
    c[:HID, C_B1] = b1
    return c


def _in_maps(x, W1, b1, Wd1, bd1, Wd2, bd2, Wd3, bd3, W2, b2, bases, gemm_mode):
    bases_np = np.float32 if gemm_mode == "f32r" else np.float16
    common = {
        "consts": _pack_consts(W1, b1, Wd1, bd1, Wd2, bd2, Wd3, bd3, W2, b2),
        "bases": np.ascontiguousarray(bases.astype(bases_np)),
    }
    maps = []
    for i in range(N_CORES):
        m = dict(common)
        m["xT"] = np.ascontiguousarray(x[i * B_LOC : (i + 1) * B_LOC].T)
        maps.append(m)
    return maps


def run(inputs: dict, gemm_mode: str = GEMM_MODE, out_mode: str = OUT_MODE,
        repeat: int = 1, **run_kwargs):
    """Shard, execute on 8 cores, gather. Returns (out, BassKernelResults)."""
    nc = _get_nc(gemm_mode, out_mode, repeat)
    in_maps = _in_maps(**{k: np.asarray(v) for k, v in inputs.items()}, gemm_mode=gemm_mode)
    res = run_bass_kernel_spmd(nc, in_maps, list(range(N_CORES)), **run_kwargs)
    shards = [np.asarray(res.results[i]["out"], dtype=np.float32) for i in range(N_CORES)]
    out = np.concatenate(shards, axis=0)
    return out, res


def kernel(**inputs) -> np.ndarray:
    out, _ = run(inputs)
    return out



# revision 3
# speedup vs baseline: 3.3540x; 3.3540x over previous
"""Trainium2 Bass kernel for nn_CoefficientDecoder.

reference computation (all f32):
    h = relu(x @ W1.T + b1)         x:[B,256] -> h:[B,64]
    h = h @ Wd3.T + bd3             [B,64]
    h = h @ Wd2.T + bd2             [B,64]
    h = h @ Wd1.T + bd1             [B,64]
    z = h @ W2.T + b2               [B,512]
    out = z @ bases                 bases:[512,4096] -> out:[B,4096]

Strategy: pure data-parallel over the batch dim across 8 NeuronCores
(B=8192 -> 1024 rows/core).

Everything after the ReLU is linear, so it is folded host-side:
    W2eff = W2@Wd1@Wd2@Wd3                     [512, 64]
    b2eff = b2 + (bd3@Wd2.T@Wd1.T + bd2@Wd1.T + bd1)@W2.T
    Beff  = W2eff.T @ bases                    [64, 4096]
    brow  = b2eff @ bases                      [4096]
    out   = relu(x@W1.T + b1) @ Beff + brow

The bias row rides along as contraction row 64: W1 gets a 65th output
channel with zero weights and bias 1 (relu(1)=1), and Beff gets brow as
row 64.  This collapses the big GEMM's contraction from K=512 to K=65,
cutting PE work 4x — which shifts the kernel from compute-bound to
DMA-bound (out is 16 MB/core in f32), so the whole pipeline runs in
fp16: x, W1, Beff loads and the out store (8 MB/core).  All fp16 keeps
rel err ~6e-4 (gate 2e-2); the DMA floor drops from ~25 MB to ~9.4 MB
per core.

Per-core schedule (all DMAs on the SP queue, in dependency order):
    load  xT j0 | beff s0-1 | xT j1 | beff s2-7      (f16, 4 DMAs + consts)
    L1    hT[65, j*512:+512] = relu(W1aug @ xT_j + b1aug)   2 j-chunks
    GEMM  mm-outer (stationary hT block reused 8x), s-inner:
          psum[128,512] = hT[:,mm*128:+128].T @ beff[:,s*512:+512]
          PSUM->SBUF f16 copies round-robin ACT/DVE/Pool
          stores per mm: [128,0:1024] after s1 (early), [128,1024:4096]
          after s7 — 16 big stores total, each 128 contiguous rows.
"""

import numpy as np

import concourse.bass as bass
import concourse.tile as tile
from concourse import bacc, mybir
from concourse.bass import ts
from concourse.bass_utils import run_bass_kernel_spmd

N_CORES = 8
B, IN_F, HID, NB, SEQ = 8192, 256, 64, 512, 4096
B_LOC = B // N_CORES            # 1024 batch rows per core
HA = HID + 1                    # 65: hidden + ones row (bias via matmul)

F32 = mybir.dt.float32
F16 = mybir.dt.float16

# kept for test.py compat; this kernel is fp16-only
GEMM_MODE = "f16"
OUT_MODE = "f16"

_CACHE = {}


def _build(gemm_mode: str = GEMM_MODE, out_mode: str = OUT_MODE, repeat: int = 1):
    nc = bacc.Bacc(
        "TRN2",
        target_bir_lowering=False,
        debug=False,
        enable_asserts=False,
        num_devices=N_CORES,
    )

    xT_d = nc.declare_dram_parameter("xT", [IN_F, B_LOC], F16, isOutput=False)
    w1c_d = nc.declare_dram_parameter("w1c", [128, 2 * HA], F16, isOutput=False)
    b1c_d = nc.declare_dram_parameter("b1c", [128, 1], F32, isOutput=False)
    beff_d = nc.declare_dram_parameter("beff", [HA, SEQ], F16, isOutput=False)
    out_d = nc.declare_dram_parameter("out", [B_LOC, SEQ], F16, isOutput=True)

    NJ = 2                  # L1 batch chunks of 512
    MM = B_LOC // 128       # 8 batch blocks for the GEMM
    SC = SEQ // 512         # 8 seq chunks

    relu = mybir.ActivationFunctionType.Relu
    copyf = mybir.ActivationFunctionType.Copy

    with tile.TileContext(nc) as tc:
        with (
            tc.tile_pool(name="const", bufs=1) as constp,
            tc.tile_pool(name="data", bufs=1) as datap,
            tc.tile_pool(name="outsb", bufs=3) as outsbp,
            tc.tile_pool(name="h_ps", bufs=2, space="PSUM") as hpp,
            tc.tile_pool(name="o_ps", bufs=6, space="PSUM") as opp,
        ):
            def body():
                w1 = constp.tile([128, 2, HA], F16, tag="w1")
                b1sb = constp.tile([128, 1], F32, tag="b1")
                xsb = datap.tile([128, 2, B_LOC], F16, tag="x")
                beff = datap.tile([HA, SEQ], F16, tag="beff")
                hT = datap.tile([HA, B_LOC], F16, tag="hT")

                xT_pkn = xT_d.rearrange("(k p) n -> p k n", p=128)
                w1_pk = w1c_d.rearrange("p (k m) -> p k m", k=2)

                # consts on the ACT queue (parallel DGE with SP's x load)
                nc.scalar.dma_start(w1[:], w1_pk[:])
                nc.scalar.dma_start(b1sb[:], b1c_d[:])
                # data loads on SP, in consumption order
                nc.sync.dma_start(xsb[:, :, 0:512], xT_pkn[:, :, 0:512])
                nc.sync.dma_start(beff[:, 0:1024], beff_d[:, 0:1024])
                nc.sync.dma_start(xsb[:, :, 512:1024], xT_pkn[:, :, 512:1024])
                nc.sync.dma_start(beff[:, 1024:4096], beff_d[:, 1024:4096])

                # L1: hT[65, 1024] = relu(W1aug @ xT + b1aug), row 64 == 1.0
                for j in range(NJ):
                    hp = hpp.tile([HA, 512], F32, tag="h")
                    for k in range(2):
                        nc.tensor.matmul(
                            hp[:],
                            w1[:, k, :],
                            xsb[:, k, ts(j, 512)],
                            start=(k == 0),
                            stop=(k == 1),
                        )
                    nc.scalar.activation(
                        hT[:, ts(j, 512)], hp[:], relu, bias=b1sb[:HA, :]
                    )

                # GEMM: out[mm*128:+128, :] = hT[:, mm-block].T @ beff
                for mm in range(MM):
                    osb = outsbp.tile([128, SEQ], F16, tag="osb")
                    for s in range(SC):
                        op = opp.tile([128, 512], F32, tag="op")
                        nc.tensor.matmul(
                            op[:],
                            hT[:, ts(mm, 128)],
                            beff[:, ts(s, 512)],
                            start=True,
                            stop=True,
                        )
                        if (mm * SC + s) % 2 == 0:
                            nc.scalar.activation(osb[:, ts(s, 512)], op[:], copyf)
                        else:
                            nc.vector.tensor_copy(osb[:, ts(s, 512)], op[:])
                        if s == 1:
                            nc.sync.dma_start(
                                out_d[ts(mm, 128), 0:1024], osb[:, 0:1024]
                            )
                    nc.sync.dma_start(
                        out_d[ts(mm, 128), 1024:4096], osb[:, 1024:4096]
                    )

            if repeat == 1:
                body()
            else:
                with tc.For_i(0, repeat, 1):
                    body()

    nc.compile()
    return nc


def _get_nc(gemm_mode: str = GEMM_MODE, out_mode: str = OUT_MODE, repeat: int = 1):
    key = (gemm_mode, out_mode, repeat)
    if key not in _CACHE:
        _CACHE[key] = _build(gemm_mode, out_mode, repeat)
    return _CACHE[key]


def _fold(W1, b1, Wd1, bd1, Wd2, bd2, Wd3, bd3, W2, b2, bases):
    f8 = np.float64
    W2eff = W2.astype(f8) @ Wd1.astype(f8) @ Wd2.astype(f8) @ Wd3.astype(f8)
    b2eff = b2.astype(f8) + (
        bd3.astype(f8) @ Wd2.astype(f8).T @ Wd1.astype(f8).T
        + bd2.astype(f8) @ Wd1.astype(f8).T
        + bd1.astype(f8)
    ) @ W2.astype(f8).T
    beff = np.empty((HA, SEQ), np.float16)
    beff[:HID] = (W2eff.T @ bases.astype(f8)).astype(np.float16)
    beff[HID] = (b2eff @ bases.astype(f8)).astype(np.float16)

    # W1aug: 65th output channel with zero weights + bias 1 -> relu==1.0
    w1c = np.zeros((128, 2 * HA), np.float16)
    W1T = W1.T.astype(np.float16)          # [256, 64]
    w1c[:, 0:HID] = W1T[:128]
    w1c[:, HA : HA + HID] = W1T[128:]
    b1c = np.zeros((128, 1), np.float32)
    b1c[:HID, 0] = b1
    b1c[HID, 0] = 1.0
    return w1c, b1c, beff


def _in_maps(x, W1, b1, Wd1, bd1, Wd2, bd2, Wd3, bd3, W2, b2, bases,
             gemm_mode=GEMM_MODE):
    w1c, b1c, beff = _fold(W1, b1, Wd1, bd1, Wd2, bd2, Wd3, bd3, W2, b2, bases)
    common = {"w1c": w1c, "b1c": b1c, "beff": beff}
    maps = []
    for i in range(N_CORES):
        m = dict(common)
        m["xT"] = np.ascontiguousarray(
            x[i * B_LOC : (i + 1) * B_LOC].T.astype(np.float16)
        )
        maps.append(m)
    return maps


def run(inputs: dict, gemm_mode: str = GEMM_MODE, out_mode: str = OUT_MODE,
        repeat: int = 1, **run_kwargs):
    """Shard, execute on 8 cores, gather. Returns (out, BassKernelResults)."""
    nc = _get_nc(gemm_mode, out_mode, repeat)
    in_maps = _in_maps(**{k: np.asarray(v) for k, v in inputs.items()},
                       gemm_mode=gemm_mode)
    res = run_bass_kernel_spmd(nc, in_maps, list(range(N_CORES)), **run_kwargs)
    shards = [np.asarray(res.results[i]["out"], dtype=np.float32)
              for i in range(N_CORES)]
    out = np.concatenate(shards, axis=0)
    return out, res


def kernel(**inputs) -> np.ndarray:
    out, _ = run(inputs)
    return out


# revision 7
# speedup vs baseline: 5.0075x; 1.4930x over previous
"""Trainium2 Bass kernel for nn_CoefficientDecoder.

reference computation (all f32):
    h = relu(x @ W1.T + b1)         x:[B,256] -> h:[B,64]
    h = h @ Wd3.T + bd3             [B,64]
    h = h @ Wd2.T + bd2             [B,64]
    h = h @ Wd1.T + bd1             [B,64]
    z = h @ W2.T + b2               [B,512]
    out = z @ bases                 bases:[512,4096] -> out:[B,4096]

Strategy: pure data-parallel over the batch dim across 8 NeuronCores
(B=8192 -> 1024 rows/core).

Everything after the ReLU is linear, so it is folded host-side:
    W2eff = W2@Wd1@Wd2@Wd3                     [512, 64]
    b2eff = b2 + (bd3@Wd2.T@Wd1.T + bd2@Wd1.T + bd1)@W2.T
    Beff  = W2eff.T @ bases                    [64, 4096]
    brow  = b2eff @ bases                      [4096]
    out   = relu(x@W1.T + b1) @ Beff + brow

The bias row rides along as contraction row 64: W1 gets a 65th output
channel with zero weights and bias 1 (relu(1)=1), and Beff gets brow as
row 64.  This collapses the big GEMM's contraction from K=512 to K=65,
cutting PE work 4x — which shifts the kernel from compute-bound to
DMA-bound (out is 16 MB/core in f32), so the whole pipeline runs in
fp16: x, W1, Beff loads and the out store (8 MB/core).  All fp16 keeps
rel err ~6e-4 (gate 2e-2); the DMA floor drops from ~25 MB to ~9.4 MB
per core.

Per-core schedule (all DMAs on the SP queue, in dependency order):
    load  xT j0 | beff s0-1 | xT j1 | beff s2-7      (f16, 4 DMAs + consts)
    L1    hT[65, j*512:+512] = relu(W1aug @ xT_j + b1aug)   2 j-chunks
    GEMM  mm-outer (stationary hT block reused 8x), s-inner:
          psum[128,512] = hT[:,mm*128:+128].T @ beff[:,s*512:+512]
          PSUM->SBUF f16 copies round-robin ACT/DVE/Pool
          stores per mm: [128,0:1024] after s1 (early), [128,1024:4096]
          after s7 — 16 big stores total, each 128 contiguous rows.
"""

import numpy as np

import concourse.bass as bass
import concourse.tile as tile
from concourse import bacc, mybir
from concourse.bass import ts
from concourse.bass_utils import run_bass_kernel_spmd

N_CORES = 8
B, IN_F, HID, NB, SEQ = 8192, 256, 64, 512, 4096
B_LOC = B // N_CORES            # 1024 batch rows per core
HA = HID + 1                    # 65: hidden + ones row (bias via matmul)

F32 = mybir.dt.float32
F16 = mybir.dt.float16

# kept for test.py compat; this kernel is fp16-only
GEMM_MODE = "f16"
OUT_MODE = "f16"

_CACHE = {}


def _build(gemm_mode: str = GEMM_MODE, out_mode: str = OUT_MODE, repeat: int = 1):
    nc = bacc.Bacc(
        "TRN2",
        target_bir_lowering=False,
        debug=False,
        enable_asserts=False,
        num_devices=N_CORES,
    )

    xT_d = nc.declare_dram_parameter("xT", [IN_F, B_LOC], F16, isOutput=False)
    w1c_d = nc.declare_dram_parameter("w1c", [128, 2 * HA], F16, isOutput=False)
    b1c_d = nc.declare_dram_parameter("b1c", [128, 1], F32, isOutput=False)
    beff_d = nc.declare_dram_parameter("beff", [HA, SEQ], F16, isOutput=False)
    out_d = nc.declare_dram_parameter("out", [B_LOC, SEQ], F16, isOutput=True)

    NJ = 2                  # L1 batch chunks of 512
    MM = B_LOC // 128       # 8 batch blocks for the GEMM
    SC = SEQ // 512         # 8 seq chunks

    relu = mybir.ActivationFunctionType.Relu
    copyf = mybir.ActivationFunctionType.Copy

    with tile.TileContext(nc) as tc:
        with (
            tc.tile_pool(name="const", bufs=1) as constp,
            tc.tile_pool(name="data", bufs=1) as datap,
            tc.tile_pool(name="outsb", bufs=3) as outsbp,
            tc.tile_pool(name="h_ps", bufs=2, space="PSUM") as hpp,
            tc.tile_pool(name="o_ps", bufs=3, space="PSUM") as opp,
        ):
            def body():
                w1 = constp.tile([128, 2, HA], F16, tag="w1")
                b1sb = constp.tile([128, 1], F32, tag="b1")
                xsb = datap.tile([128, 2, B_LOC], F16, tag="x")
                beff = datap.tile([HA, SEQ], F16, tag="beff")
                hT = datap.tile([HA, B_LOC], F16, tag="hT")

                xT_pkn = xT_d.rearrange("(k p) n -> p k n", p=128)
                w1_pk = w1c_d.rearrange("p (k m) -> p k m", k=2)

                # consts on the ACT queue (parallel DGE with SP's x load;
                # ACT is idle until the loads land, so its seq time is free)
                nc.scalar.dma_start(w1[:], w1_pk[:])
                nc.scalar.dma_start(b1sb[:], b1c_d[:])
                # data loads on SP, in consumption order
                nc.sync.dma_start(xsb[:, :, 0:512], xT_pkn[:, :, 0:512])
                nc.sync.dma_start(beff[:, 0:1024], beff_d[:, 0:1024])
                nc.sync.dma_start(xsb[:, :, 512:1024], xT_pkn[:, :, 512:1024])
                nc.sync.dma_start(beff[:, 1024:4096], beff_d[:, 1024:4096])

                # L1: hT[65, 1024] = relu(W1aug @ xT + b1aug), row 64 == 1.0
                for j in range(NJ):
                    hp = hpp.tile([HA, 512], F32, tag="h")
                    for k in range(2):
                        nc.tensor.matmul(
                            hp[:],
                            w1[:, k, :],
                            xsb[:, k, ts(j, 512)],
                            start=(k == 0),
                            stop=(k == 1),
                        )
                    nc.scalar.activation(
                        hT[:, ts(j, 512)], hp[:], relu, bias=b1sb[:HA, :]
                    )

                # GEMM: out[mm*128:+128, :] = hT[:, mm-block].T @ beff
                # PSUM drains in 2-bank [128, 1024] chunks, ACT/DVE weighted
                # ~17:15 (ACT is 1.2 GHz vs DVE 0.96, but also does the relus)
                NH = SEQ // 1024        # 4 drain chunks per mm block
                drain_ctr = 0
                for mm in range(MM):
                    osb = outsbp.tile([128, SEQ], F16, tag="osb")
                    for sh in range(NH):
                        op = opp.tile([128, 1024], F32, tag="op")
                        for t in range(2):
                            nc.tensor.matmul(
                                op[:, ts(t, 512)],
                                hT[:, ts(mm, 128)],
                                beff[:, ts(2 * sh + t, 512)],
                                start=True,
                                stop=True,
                            )
                        use_act = (drain_ctr * 17) // 32 != ((drain_ctr + 1) * 17) // 32
                        drain_ctr += 1
                        if use_act:
                            nc.scalar.activation(osb[:, ts(sh, 1024)], op[:], copyf)
                        else:
                            nc.vector.tensor_copy(osb[:, ts(sh, 1024)], op[:])
                        if sh == 0:
                            nc.sync.dma_start(
                                out_d[ts(mm, 128), 0:1024], osb[:, 0:1024]
                            )
                    nc.sync.dma_start(
                        out_d[ts(mm, 128), 1024:4096], osb[:, 1024:4096]
                    )

            if repeat == 1:
                body()
            else:
                with tc.For_i(0, repeat, 1):
                    body()

    nc.compile()
    return nc


def _get_nc(gemm_mode: str = GEMM_MODE, out_mode: str = OUT_MODE, repeat: int = 1):
    key = (gemm_mode, out_mode, repeat)
    if key not in _CACHE:
        _CACHE[key] = _build(gemm_mode, out_mode, repeat)
    return _CACHE[key]


def _fold(W1, b1, Wd1, bd1, Wd2, bd2, Wd3, bd3, W2, b2, bases):
    f8 = np.float64
    W2eff = W2.astype(f8) @ Wd1.astype(f8) @ Wd2.astype(f8) @ Wd3.astype(f8)
    b2eff = b2.astype(f8) + (
        bd3.astype(f8) @ Wd2.astype(f8).T @ Wd1.astype(f8).T
        + bd2.astype(f8) @ Wd1.astype(f8).T
        + bd1.astype(f8)
    ) @ W2.astype(f8).T
    beff = np.empty((HA, SEQ), np.float16)
    beff[:HID] = (W2eff.T @ bases.astype(f8)).astype(np.float16)
    beff[HID] = (b2eff @ bases.astype(f8)).astype(np.float16)

    # W1aug: 65th output channel with zero weights + bias 1 -> relu==1.0
    w1c = np.zeros((128, 2 * HA), np.float16)
    W1T = W1.T.astype(np.float16)          # [256, 64]
    w1c[:, 0:HID] = W1T[:128]
    w1c[:, HA : HA + HID] = W1T[128:]
    b1c = np.zeros((128, 1), np.float32)
    b1c[:HID, 0] = b1
    b1c[HID, 0] = 1.0
    return w1c, b1c, beff


def _in_maps(x, W1, b1, Wd1, bd1, Wd2, bd2, Wd3, bd3, W2, b2, bases,
             gemm_mode=GEMM_MODE):
    w1c, b1c, beff = _fold(W1, b1, Wd1, bd1, Wd2, bd2, Wd3, bd3, W2, b2, bases)
    common = {"w1c": w1c, "b1c": b1c, "beff": beff}
    maps = []
    for i in range(N_CORES):
        m = dict(common)
        m["xT"] = np.ascontiguousarray(
            x[i * B_LOC : (i + 1) * B_LOC].T.astype(np.float16)
        )
        maps.append(m)
    return maps


def run(inputs: dict, gemm_mode: str = GEMM_MODE, out_mode: str = OUT_MODE,
        repeat: int = 1, **run_kwargs):
    """Shard, execute on 8 cores, gather. Returns (out, BassKernelResults)."""
    nc = _get_nc(gemm_mode, out_mode, repeat)
    in_maps = _in_maps(**{k: np.asarray(v) for k, v in inputs.items()},
                       gemm_mode=gemm_mode)
    res = run_bass_kernel_spmd(nc, in_maps, list(range(N_CORES)), **run_kwargs)
    shards = [np.asarray(res.results[i]["out"], dtype=np.float32)
              for i in range(N_CORES)]
    out = np.concatenate(shards, axis=0)
    return out, res


def kernel(**inputs) -> np.ndarray:
    out, _ = run(inputs)
    return out


# revision 11
# speedup vs baseline: 6.2971x; 1.2576x over previous
"""Trainium2 Bass kernel for nn_CoefficientDecoder.

reference computation (all f32):
    h = relu(x @ W1.T + b1)         x:[B,256] -> h:[B,64]
    h = h @ Wd3.T + bd3             [B,64]
    h = h @ Wd2.T + bd2             [B,64]
    h = h @ Wd1.T + bd1             [B,64]
    z = h @ W2.T + b2               [B,512]
    out = z @ bases                 bases:[512,4096] -> out:[B,4096]

Strategy: pure data-parallel over the batch dim across 8 NeuronCores
(B=8192 -> 1024 rows/core).

Everything after the ReLU is linear, so it is folded host-side:
    W2eff = W2@Wd1@Wd2@Wd3                     [512, 64]
    b2eff = b2 + (bd3@Wd2.T@Wd1.T + bd2@Wd1.T + bd1)@W2.T
    Beff  = W2eff.T @ bases                    [64, 4096]
    brow  = b2eff @ bases                      [4096]
    out   = relu(x@W1.T + b1) @ Beff + brow

The bias row rides along as contraction row 64: W1 gets a 65th output
channel with zero weights and bias 1 (relu(1)=1), and Beff gets brow as
row 64.  This collapses the big GEMM's contraction from K=512 to K=65,
cutting PE work 4x — which shifts the kernel from compute-bound to
DMA-bound (out is 16 MB/core in f32), so the whole pipeline runs in
fp16: x, W1, Beff loads and the out store (8 MB/core).  All fp16 keeps
rel err ~6e-4 (gate 2e-2); the DMA floor drops from ~25 MB to ~9.4 MB
per core.

Per-core schedule (all DMAs on the SP queue, in dependency order):
    load  xT j0 | beff s0-1 | xT j1 | beff s2-7      (f16, 4 DMAs + consts)
    L1    hT[65, j*512:+512] = relu(W1aug @ xT_j + b1aug)   2 j-chunks
    GEMM  mm-outer (stationary hT block reused 8x), s-inner:
          psum[128,512] = hT[:,mm*128:+128].T @ beff[:,s*512:+512]
          PSUM->SBUF f16 copies round-robin ACT/DVE/Pool
          stores per mm: [128,0:1024] after s1 (early), [128,1024:4096]
          after s7 — 16 big stores total, each 128 contiguous rows.
"""

import numpy as np

import concourse.bass as bass
import concourse.tile as tile
from concourse import bacc, mybir
from concourse.bass import ts
from concourse.bass_utils import run_bass_kernel_spmd

N_CORES = 8
B, IN_F, HID, NB, SEQ = 8192, 256, 64, 512, 4096
B_LOC = B // N_CORES            # 1024 batch rows per core
HA = HID + 1                    # 65: hidden + ones row (bias via matmul)

F32 = mybir.dt.float32
F16 = mybir.dt.float16

# kept for test.py compat; this kernel is fp16-only
GEMM_MODE = "f16"
OUT_MODE = "f16"

_CACHE = {}


def _build(gemm_mode: str = GEMM_MODE, out_mode: str = OUT_MODE, repeat: int = 1):
    out_dt = F32 if out_mode == "f32" else F16
    nc = bacc.Bacc(
        "TRN2",
        target_bir_lowering=False,
        debug=False,
        enable_asserts=False,
        num_devices=N_CORES,
    )

    xT_d = nc.declare_dram_parameter("xT", [IN_F, B_LOC], F16, isOutput=False)
    w1c_d = nc.declare_dram_parameter("w1c", [128, 2 * HA], F16, isOutput=False)
    b1c_d = nc.declare_dram_parameter("b1c", [128, 1], F32, isOutput=False)
    beff_d = nc.declare_dram_parameter("beff", [HA, SEQ], F16, isOutput=False)
    out_d = nc.declare_dram_parameter("out", [B_LOC, SEQ], out_dt, isOutput=True)

    NJ = 2                  # L1 batch chunks of 512
    MM = B_LOC // 128       # 8 batch blocks for the GEMM
    SC = SEQ // 512         # 8 seq chunks

    relu = mybir.ActivationFunctionType.Relu
    copyf = mybir.ActivationFunctionType.Copy

    with tile.TileContext(nc) as tc:
        with (
            tc.tile_pool(name="const", bufs=1) as constp,
            tc.tile_pool(name="data", bufs=1) as datap,
            tc.tile_pool(name="outsb", bufs=3) as outsbp,
            tc.tile_pool(name="h_ps", bufs=2, space="PSUM") as hpp,
            tc.tile_pool(name="o_ps", bufs=3, space="PSUM") as opp,
        ):
            def body():
                w1 = constp.tile([128, 2, HA], F16, tag="w1")
                b1sb = constp.tile([128, 1], F32, tag="b1")
                xsb = datap.tile([128, 2, B_LOC], F16, tag="x")
                beff = datap.tile([HA, SEQ], F16, tag="beff")
                hT = datap.tile([HA, B_LOC], F16, tag="hT")

                xT_pkn = xT_d.rearrange("(k p) n -> p k n", p=128)
                w1_pk = w1c_d.rearrange("p (k m) -> p k m", k=2)

                # consts + beff on the ACT queue (parallel DGE with SP's x
                # loads; ACT is idle until the loads land, so its seq time
                # is free), x on SP
                nc.scalar.dma_start(w1[:], w1_pk[:])
                nc.scalar.dma_start(b1sb[:], b1c_d[:])
                nc.sync.dma_start(xsb[:, :, 0:512], xT_pkn[:, :, 0:512])
                nc.scalar.dma_start(beff[:, 0:1024], beff_d[:, 0:1024])
                nc.sync.dma_start(xsb[:, :, 512:1024], xT_pkn[:, :, 512:1024])
                nc.scalar.dma_start(beff[:, 1024:4096], beff_d[:, 1024:4096])

                # L1: hT[65, 1024] = relu(W1aug @ xT + b1aug), row 64 == 1.0
                for j in range(NJ):
                    hp = hpp.tile([HA, 512], F32, tag="h")
                    for k in range(2):
                        nc.tensor.matmul(
                            hp[:],
                            w1[:, k, :],
                            xsb[:, k, ts(j, 512)],
                            start=(k == 0),
                            stop=(k == 1),
                        )
                    nc.scalar.activation(
                        hT[:, ts(j, 512)], hp[:], relu, bias=b1sb[:HA, :]
                    )

                # GEMM: out[mm*128:+128, :] = hT[:, mm-block].T @ beff
                # PSUM drains in 2-bank [128, 1024] chunks, ACT/DVE weighted
                # ~17:15 (ACT is 1.2 GHz vs DVE 0.96, but also does the relus)
                NH = SEQ // 1024        # 4 drain chunks per mm block
                drain_ctr = 0
                for mm in range(MM):
                    osb = outsbp.tile([128, SEQ], out_dt, tag="osb")
                    for sh in range(NH):
                        op = opp.tile([128, 1024], F32, tag="op")
                        for t in range(2):
                            nc.tensor.matmul(
                                op[:, ts(t, 512)],
                                hT[:, ts(mm, 128)],
                                beff[:, ts(2 * sh + t, 512)],
                                start=True,
                                stop=True,
                            )
                        use_act = (drain_ctr * 17) // 32 != ((drain_ctr + 1) * 17) // 32
                        drain_ctr += 1
                        if use_act:
                            nc.scalar.activation(osb[:, ts(sh, 1024)], op[:], copyf)
                        else:
                            nc.vector.tensor_copy(osb[:, ts(sh, 1024)], op[:])
                        if sh == 0:
                            st_eng = nc.sync if mm % 2 == 0 else nc.gpsimd
                            st_eng.dma_start(
                                out_d[ts(mm, 128), 0:1024], osb[:, 0:1024]
                            )
                    st_eng = nc.sync if mm % 2 == 0 else nc.gpsimd
                    st_eng.dma_start(
                        out_d[ts(mm, 128), 1024:4096], osb[:, 1024:4096]
                    )

            if repeat == 1:
                body()
            else:
                with tc.For_i(0, repeat, 1):
                    body()

    nc.compile()
    return nc


def _get_nc(gemm_mode: str = GEMM_MODE, out_mode: str = OUT_MODE, repeat: int = 1):
    key = (gemm_mode, out_mode, repeat)
    if key not in _CACHE:
        _CACHE[key] = _build(gemm_mode, out_mode, repeat)
    return _CACHE[key]


def _fold(W1, b1, Wd1, bd1, Wd2, bd2, Wd3, bd3, W2, b2, bases):
    f8 = np.float64
    W2eff = W2.astype(f8) @ Wd1.astype(f8) @ Wd2.astype(f8) @ Wd3.astype(f8)
    b2eff = b2.astype(f8) + (
        bd3.astype(f8) @ Wd2.astype(f8).T @ Wd1.astype(f8).T
        + bd2.astype(f8) @ Wd1.astype(f8).T
        + bd1.astype(f8)
    ) @ W2.astype(f8).T
    beff = np.empty((HA, SEQ), np.float16)
    beff[:HID] = (W2eff.T @ bases.astype(f8)).astype(np.float16)
    beff[HID] = (b2eff @ bases.astype(f8)).astype(np.float16)

    # W1aug: 65th output channel with zero weights + bias 1 -> relu==1.0
    w1c = np.zeros((128, 2 * HA), np.float16)
    W1T = W1.T.astype(np.float16)          # [256, 64]
    w1c[:, 0:HID] = W1T[:128]
    w1c[:, HA : HA + HID] = W1T[128:]
    b1c = np.zeros((128, 1), np.float32)
    b1c[:HID, 0] = b1
    b1c[HID, 0] = 1.0
    return w1c, b1c, beff


def _in_maps(x, W1, b1, Wd1, bd1, Wd2, bd2, Wd3, bd3, W2, b2, bases,
             gemm_mode=GEMM_MODE):
    w1c, b1c, beff = _fold(W1, b1, Wd1, bd1, Wd2, bd2, Wd3, bd3, W2, b2, bases)
    common = {"w1c": w1c, "b1c": b1c, "beff": beff}
    maps = []
    for i in range(N_CORES):
        m = dict(common)
        m["xT"] = np.ascontiguousarray(
            x[i * B_LOC : (i + 1) * B_LOC].T.astype(np.float16)
        )
        maps.append(m)
    return maps


def run(inputs: dict, gemm_mode: str = GEMM_MODE, out_mode: str = OUT_MODE,
        repeat: int = 1, **run_kwargs):
    """Shard, execute on 8 cores, gather. Returns (out, BassKernelResults)."""
    nc = _get_nc(gemm_mode, out_mode, repeat)
    in_maps = _in_maps(**{k: np.asarray(v) for k, v in inputs.items()},
                       gemm_mode=gemm_mode)
    res = run_bass_kernel_spmd(nc, in_maps, list(range(N_CORES)), **run_kwargs)
    shards = [np.asarray(res.results[i]["out"], dtype=np.float32)
              for i in range(N_CORES)]
    out = np.concatenate(shards, axis=0)
    return out, res


def kernel(**inputs) -> np.ndarray:
    out, _ = run(inputs)
    return out


# revision 13
# speedup vs baseline: 20.1357x; 3.1976x over previous
"""Trainium2 Bass kernel for nn_CoefficientDecoder.

reference computation (all f32):
    h = relu(x @ W1.T + b1)         x:[B,256] -> h:[B,64]
    h = h @ Wd3.T + bd3             [B,64]
    h = h @ Wd2.T + bd2             [B,64]
    h = h @ Wd1.T + bd1             [B,64]
    z = h @ W2.T + b2               [B,512]
    out = z @ bases                 bases:[512,4096] -> out:[B,4096]

Strategy: pure data-parallel over the batch dim across 8 NeuronCores
(B=8192 -> 1024 rows/core).

Everything after the ReLU is linear, so it is folded host-side:
    W2eff = W2@Wd1@Wd2@Wd3                     [512, 64]
    b2eff = b2 + (bd3@Wd2.T@Wd1.T + bd2@Wd1.T + bd1)@W2.T
    Beff  = W2eff.T @ bases                    [64, 4096]
    brow  = b2eff @ bases                      [4096]
    out   = relu(x@W1.T + b1) @ Beff + brow

The bias row rides along as contraction row 64: W1 gets a 65th output
channel with zero weights and bias 1 (relu(1)=1), and Beff gets brow as
row 64.  This collapses the big GEMM's contraction from K=512 to K=65,
cutting PE work 4x — which shifts the kernel from compute-bound to
DMA-bound (out is 16 MB/core in f32), so the whole pipeline runs in
fp16: x, W1, Beff loads and the out store (8 MB/core).  All fp16 keeps
rel err ~6e-4 (gate 2e-2); the DMA floor drops from ~25 MB to ~9.4 MB
per core.

Per-core schedule (all DMAs on the SP queue, in dependency order):
    load  xT j0 | beff s0-1 | xT j1 | beff s2-7      (f16, 4 DMAs + consts)
    L1    hT[65, j*512:+512] = relu(W1aug @ xT_j + b1aug)   2 j-chunks
    GEMM  mm-outer (stationary hT block reused 8x), s-inner:
          psum[128,512] = hT[:,mm*128:+128].T @ beff[:,s*512:+512]
          PSUM->SBUF f16 copies round-robin ACT/DVE/Pool
          stores per mm: [128,0:1024] after s1 (early), [128,1024:4096]
          after s7 — 16 big stores total, each 128 contiguous rows.
"""

import numpy as np

import concourse.bass as bass
import concourse.tile as tile
from concourse import bacc, mybir
from concourse.bass import ts
from concourse.bass_utils import run_bass_kernel_spmd

N_CORES = 8
B, IN_F, HID, NB, SEQ = 8192, 256, 64, 512, 4096
B_LOC = B // N_CORES            # 1024 batch rows per core
HA = HID + 1                    # 65: hidden + ones row (bias via matmul)

F32 = mybir.dt.float32
F16 = mybir.dt.float16

# kept for test.py compat; this kernel is fp16-only
GEMM_MODE = "f16"
OUT_MODE = "f16"

_CACHE = {}


def _build(gemm_mode: str = GEMM_MODE, out_mode: str = OUT_MODE, repeat: int = 1):
    out_dt = F32 if out_mode == "f32" else F16
    nc = bacc.Bacc(
        "TRN2",
        target_bir_lowering=False,
        debug=False,
        enable_asserts=False,
        num_devices=N_CORES,
    )

    xT_d = nc.declare_dram_parameter("xT", [IN_F, B_LOC], F16, isOutput=False)
    w1c_d = nc.declare_dram_parameter("w1c", [128, 2 * HA], F16, isOutput=False)
    b1c_d = nc.declare_dram_parameter("b1c", [128, 1], F32, isOutput=False)
    beff_d = nc.declare_dram_parameter("beff", [HA, SEQ], F16, isOutput=False)
    out_d = nc.declare_dram_parameter("out", [B_LOC, SEQ], out_dt, isOutput=True)

    NJ = 2                  # L1 batch chunks of 512
    MM = B_LOC // 128       # 8 batch blocks for the GEMM
    SC = SEQ // 512         # 8 seq chunks

    relu = mybir.ActivationFunctionType.Relu
    copyf = mybir.ActivationFunctionType.Copy

    with tile.TileContext(nc) as tc:
        with (
            tc.tile_pool(name="const", bufs=1) as constp,
            tc.tile_pool(name="data", bufs=1) as datap,
            tc.tile_pool(name="outsb", bufs=3) as outsbp,
            tc.tile_pool(name="h_ps", bufs=2, space="PSUM") as hpp,
            tc.tile_pool(name="o_ps", bufs=3, space="PSUM") as opp,
        ):
            def body():
                w1 = constp.tile([128, 2, HA], F16, tag="w1")
                b1sb = constp.tile([128, 1], F32, tag="b1")
                xsb = datap.tile([128, 2, B_LOC], F16, tag="x")
                beff = datap.tile([HA, SEQ], F16, tag="beff")
                hT = datap.tile([HA, B_LOC], F16, tag="hT")

                xT_pkn = xT_d.rearrange("(k p) n -> p k n", p=128)
                w1_pk = w1c_d.rearrange("p (k m) -> p k m", k=2)

                # consts on the ACT queue (ACT idle until loads land, so
                # its seq time is free); x + beff stream on SP in
                # consumption order, keeping ACT's sequencer for drains
                nc.scalar.dma_start(w1[:], w1_pk[:])
                nc.scalar.dma_start(b1sb[:], b1c_d[:])
                nc.sync.dma_start(xsb[:, :, 0:512], xT_pkn[:, :, 0:512])
                nc.sync.dma_start(beff[:, 0:1024], beff_d[:, 0:1024])
                nc.sync.dma_start(xsb[:, :, 512:1024], xT_pkn[:, :, 512:1024])
                nc.sync.dma_start(beff[:, 1024:4096], beff_d[:, 1024:4096])

                # L1: hT[65, 1024] = relu(W1aug @ xT + b1aug), row 64 == 1.0
                for j in range(NJ):
                    hp = hpp.tile([HA, 512], F32, tag="h")
                    for k in range(2):
                        nc.tensor.matmul(
                            hp[:],
                            w1[:, k, :],
                            xsb[:, k, ts(j, 512)],
                            start=(k == 0),
                            stop=(k == 1),
                        )
                    nc.scalar.activation(
                        hT[:, ts(j, 512)], hp[:], relu, bias=b1sb[:HA, :]
                    )

                # GEMM: out[mm*128:+128, :] = hT[:, mm-block].T @ beff
                # PSUM drains in 2-bank [128, 1024] chunks, ACT/DVE weighted
                # ~17:15 (ACT is 1.2 GHz vs DVE 0.96, but also does the relus)
                NH = SEQ // 1024        # 4 drain chunks per mm block
                drain_ctr = 0
                for mm in range(MM):
                    osb = outsbp.tile([128, SEQ], out_dt, tag="osb")
                    for sh in range(NH):
                        op = opp.tile([128, 1024], F32, tag="op")
                        for t in range(2):
                            nc.tensor.matmul(
                                op[:, ts(t, 512)],
                                hT[:, ts(mm, 128)],
                                beff[:, ts(2 * sh + t, 512)],
                                start=True,
                                stop=True,
                            )
                        use_act = (drain_ctr * 17) // 32 != ((drain_ctr + 1) * 17) // 32
                        drain_ctr += 1
                        if use_act:
                            nc.scalar.activation(osb[:, ts(sh, 1024)], op[:], copyf)
                        else:
                            nc.vector.tensor_copy(osb[:, ts(sh, 1024)], op[:])
                        if sh == 0 and mm % 2 == 0:
                            # early partial store primes the pipe (SP mms
                            # only: SWDGE's ~1us per-DMA gen cost on the
                            # Pool sequencer makes extra gpsimd DMAs dear)
                            nc.sync.dma_start(
                                out_d[ts(mm, 128), 0:1024], osb[:, 0:1024]
                            )
                    if mm % 2 == 0:
                        nc.sync.dma_start(
                            out_d[ts(mm, 128), 1024:4096], osb[:, 1024:4096]
                        )
                    else:
                        nc.gpsimd.dma_start(out_d[ts(mm, 128), :], osb[:])

            if repeat == 1:
                body()
            else:
                with tc.For_i(0, repeat, 1):
                    body()

    nc.compile()
    return nc


def _get_nc(gemm_mode: str = GEMM_MODE, out_mode: str = OUT_MODE, repeat: int = 1):
    key = (gemm_mode, out_mode, repeat)
    if key not in _CACHE:
        _CACHE[key] = _build(gemm_mode, out_mode, repeat)
    return _CACHE[key]


def _fold(W1, b1, Wd1, bd1, Wd2, bd2, Wd3, bd3, W2, b2, bases):
    f8 = np.float64
    W2eff = W2.astype(f8) @ Wd1.astype(f8) @ Wd2.astype(f8) @ Wd3.astype(f8)
    b2eff = b2.astype(f8) + (
        bd3.astype(f8) @ Wd2.astype(f8).T @ Wd1.astype(f8).T
        + bd2.astype(f8) @ Wd1.astype(f8).T
        + bd1.astype(f8)
    ) @ W2.astype(f8).T
    beff = np.empty((HA, SEQ), np.float16)
    beff[:HID] = (W2eff.T @ bases.astype(f8)).astype(np.float16)
    beff[HID] = (b2eff @ bases.astype(f8)).astype(np.float16)

    # W1aug: 65th output channel with zero weights + bias 1 -> relu==1.0
    w1c = np.zeros((128, 2 * HA), np.float16)
    W1T = W1.T.astype(np.float16)          # [256, 64]
    w1c[:, 0:HID] = W1T[:128]
    w1c[:, HA : HA + HID] = W1T[128:]
    b1c = np.zeros((128, 1), np.float32)
    b1c[:HID, 0] = b1
    b1c[HID, 0] = 1.0
    return w1c, b1c, beff


def _in_maps(x, W1, b1, Wd1, bd1, Wd2, bd2, Wd3, bd3, W2, b2, bases,
             gemm_mode=GEMM_MODE):
    w1c, b1c, beff = _fold(W1, b1, Wd1, bd1, Wd2, bd2, Wd3, bd3, W2, b2, bases)
    common = {"w1c": w1c, "b1c": b1c, "beff": beff}
    maps = []
    for i in range(N_CORES):
        m = dict(common)
        m["xT"] = np.ascontiguousarray(
            x[i * B_LOC : (i + 1) * B_LOC].T.astype(np.float16)
        )
        maps.append(m)
    return maps


def run(inputs: dict, gemm_mode: str = GEMM_MODE, out_mode: str = OUT_MODE,
        repeat: int = 1, **run_kwargs):
    """Shard, execute on 8 cores, gather. Returns (out, BassKernelResults)."""
    nc = _get_nc(gemm_mode, out_mode, repeat)
    in_maps = _in_maps(**{k: np.asarray(v) for k, v in inputs.items()},
                       gemm_mode=gemm_mode)
    res = run_bass_kernel_spmd(nc, in_maps, list(range(N_CORES)), **run_kwargs)
    shards = [np.asarray(res.results[i]["out"], dtype=np.float32)
              for i in range(N_CORES)]
    out = np.concatenate(shards, axis=0)
    return out, res


def kernel(**inputs) -> np.ndarray:
    out, _ = run(inputs)
    return out
